# revision 40
# baseline (speedup 1.0000x reference)
"""Trainium2 Bass kernel for nn_F0Collisions: batched Chang-Cooper implicit
Fokker-Planck solve, 16384 x 512, data-parallel over rows across 8 cores.

Each row's tridiagonal system depends on the row only through one scalar
lam = Sg*S4/(6*DV*S2^2).  The Thomas-factorization profiles alpha~(lam)
(premultiply-free forward coefficient via dp = beta*e), beta(lam) and
cp(lam) are smooth in lam and are interpolated from Chebyshev-coefficient
tables with one PE matmul per table per 128-row tile.  Host preprocessing
computes the per-row Chebyshev basis (extending the baseline's host-side
lam calibration) and ships it as a split-fp16 stacked operand
PT = [Ph | Ph | Pl]; a single K=24 fp16 matmul against the matching
split-fp16 table stack [Kh; Kl; Kh] reproduces f32-accurate profiles.

Device work per 128x512 tile:
  - TensorE: 3 table matmuls ([24,128]^T fp16 x [24,512] fp16 -> PSUM f32),
    each into its own single-bank PSUM pool so the scans' semaphore waits
    are per-table, not per-tile.
  - ScalarE: copies beta from PSUM to SBUF (keeps the premultiply's
    operands off PSUM, which decontends the DVE scans).
  - VectorE (the bottleneck, ~47us busy, zero idle): forward scan
    e = scan(alpha~, f) on the raw f32 f, premultiply dp = e*beta, and a
    reversed backward scan x = scan(-cp_rev, dp_rev).  Scan state is fp32;
    the premultiply+backward scan are fused across tile PAIRS
    (cp[:, -1] = 0 exactly, so the reversed 1024-wide pair scan
    auto-resets at the tile boundary; the cp pair table is stored in
    swapped order to match the reversed stream).
  - Sync queue: constants + f loads, issued up front so transfers stream
    ahead of compute; GpSimd sw-DGE queue: x stores (last pair split
    across two queues to shorten the drain).
The value path stays f32 end to end (elem-rel ~6e-5, scale-rel ~7e-6
against the reference; the harness gate is 2e-2)."""

import numpy as np

import concourse.bass as bass
import concourse.mybir as mybir
import concourse.tile as tile
from concourse import bacc
from concourse.bass_utils import run_bass_kernel_spmd

NX, NV = 16384, 512
N_CORES = 8
ROWS = NX // N_CORES          # rows per core
NT = ROWS // 128              # 128-row tiles per core
DV = 8.0 / NV
NUEE_COEFF = 2.221e-7
M = 8                         # Chebyshev terms
KS = 3 * M                    # stacked contraction (Ph*Kh, Ph*Kl, Pl*Kh)
SQ2 = float(np.sqrt(2.0))

F32 = mybir.dt.float32
FP16 = mybir.dt.float16
ALU = mybir.AluOpType
AFT = mybir.ActivationFunctionType


# ---------------------------------------------------------------- host math

def _host_weights(v):
    v = v.astype(np.float64)
    v2 = v * v
    we = (0.5 * (v[1:] + v[:-1])) ** 2 * DV / SQ2   # sqrt_eps * d_eps
    g = np.empty(NV)
    g[0] = 0.5 * we[0]
    g[-1] = 0.5 * we[-1]
    g[1:-1] = 0.5 * (we[:-1] + we[1:])
    return v2, g


def _profiles_for_lam(lam, v, dt):
    """alpha~(reparam fwd), beta, cp for a vector of lam (float64)."""
    lam = np.asarray(lam, np.float64)
    v = v.astype(np.float64)
    v2 = v * v
    v_edge = 0.5 * (v[1:] + v[:-1])
    sqrt_eps = v_edge / SQ2
    D = sqrt_eps[None, :] * lam[:, None]
    C = v_edge[None, :]
    w = C * DV / D
    delta = 1.0 / w - 1.0 / np.expm1(w)
    lo = C * delta - D / DV
    hi = C * (1.0 - delta) + D / DV
    w2 = v_edge ** 2
    w2lo, w2hi = w2 * lo, w2 * hi
    inv = 1.0 / (v2 * DV)
    Mn = lam.shape[0]
    z = np.zeros((Mn, 1))
    diagL = (np.concatenate([w2lo, z], -1) - np.concatenate([z, w2hi], -1)) * inv
    subL = np.concatenate([z, -w2lo], -1) * inv
    supL = np.concatenate([w2hi, z], -1) * inv
    k = float(dt) * NUEE_COEFF
    a, b, c = -k * subL, 1.0 - k * diagL, -k * supL
    alpha = np.zeros((Mn, NV))
    beta = np.zeros((Mn, NV))
    cp = np.zeros((Mn, NV))
    cprev = np.zeros(Mn)
    for j in range(NV):
        denom = b[:, j] - a[:, j] * cprev
        cprev = c[:, j] / denom
        cp[:, j] = cprev
        beta[:, j] = 1.0 / denom
        alpha[:, j] = -a[:, j] / denom
    at = np.zeros_like(alpha)
    at[:, 1:] = alpha[:, 1:] * beta[:, :-1] / beta[:, 1:]
    return at, beta, cp


def _split_fp16(X):
    h = X.astype(np.float16).astype(np.float64)
    l = (X - h).astype(np.float16)
    return h.astype(np.float16), l


def _build_tables_and_basis(f0x, dt, v):
    """Chebyshev tables (split fp16, K-stacked) + per-row split basis."""
    f32 = np.asarray(f0x, np.float32)
    v2, g = _host_weights(v)
    v4 = v2 * v2
    S2 = (f32 @ v2.astype(np.float32)).astype(np.float64)
    S4 = (f32 @ v4.astype(np.float32)).astype(np.float64)
    Sg = (f32 @ g.astype(np.float32)).astype(np.float64)
    lam = Sg * S4 / (6.0 * DV * S2 * S2)
    lo, hi = float(lam.min()), float(lam.max())
    span = max(hi - lo, 1e-3 * max(abs(hi), 1e-30))
    lo -= 0.20 * span
    hi += 0.20 * span
    mid = 0.5 * (lo + hi)
    half = 0.5 * (hi - lo)

    kk = np.arange(M)
    xk = np.cos(np.pi * (kk + 0.5) / M)
    at, be, cp = _profiles_for_lam(mid + half * xk, v, dt)
    T = np.cos(np.outer(np.arange(M), np.pi * (kk + 0.5) / M))
    Wc = (2.0 / M) * T
    Wc[0, :] *= 0.5
    ktab = np.zeros((KS, 3 * NV), np.float16)
    for s, prof in enumerate((at, be, -cp[:, ::-1])):
        Kc = Wc @ prof                               # [M, NV] f64
        Kh, Kl = _split_fp16(Kc)
        ktab[0:M, s * NV:(s + 1) * NV] = Kh          # pairs with Ph
        ktab[M:2 * M, s * NV:(s + 1) * NV] = Kl      # pairs with Ph
        ktab[2 * M:3 * M, s * NV:(s + 1) * NV] = Kh  # pairs with Pl

    # per-row Chebyshev basis, split fp16, stacked [Ph | Ph | Pl]
    xi = (lam - mid) / half
    P = np.empty((NX, M))
    P[:, 0] = 1.0
    P[:, 1] = xi
    for m in range(2, M):
        P[:, m] = 2 * xi * P[:, m - 1] - P[:, m - 2]
    Ph, Pl = _split_fp16(P)
    pstack = np.concatenate([Ph, Ph, Pl], axis=1)    # [NX, 24]
    return np.ascontiguousarray(ktab), pstack


# ---------------------------------------------------------------- bass build

def build_program():
    nc = bacc.Bacc("TRN2", target_bir_lowering=False, debug=False)

    fin = nc.dram_tensor("fin", [ROWS, NV], F32, kind="ExternalInput").ap()
    ktabd = nc.dram_tensor("ktab", [KS, 3 * NV], FP16, kind="ExternalInput").ap()
    ptind = nc.dram_tensor("ptin", [KS, NT * 128], FP16,
                           kind="ExternalInput").ap()
    xout = nc.dram_tensor("xout", [ROWS, NV], F32, kind="ExternalOutput").ap()

    fin_t = fin.rearrange("(t p) j -> t p j", p=128)
    xout_t = xout.rearrange("(t p) j -> t p j", p=128)

    with tile.TileContext(nc) as tc:
        with (
            tc.tile_pool(name="const", bufs=1) as cpool,
            tc.tile_pool(name="ep", bufs=3) as epool,
            tc.tile_pool(name="dpp", bufs=3) as dppool,
            tc.tile_pool(name="xp", bufs=4) as xpool,
            tc.tile_pool(name="tabap", bufs=2, space="PSUM") as tabapool,
            tc.tile_pool(name="tabbp", bufs=2, space="PSUM") as tabbpool,
            tc.tile_pool(name="tabcp", bufs=2, space="PSUM") as tabcpool,
        ):
            kt = cpool.tile([KS, 3 * NV], FP16)
            pts = cpool.tile([KS, NT * 128], FP16)
            nc.sync.dma_start(kt[:], ktabd)
            nc.sync.dma_start(pts[:], ptind)

            fall = cpool.tile([128, NT * NV], F32)
            # all f loads issued up front on the sync queue; transfers
            # stream across the DMA channels while compute proceeds
            for t in range(NT):
                nc.sync.dma_start(fall[:, t * NV:(t + 1) * NV], fin_t[t])

            # software-pipelined fronts; premultiply + backward scan fused
            # across tile pairs (cp[:, -1] = 0 exactly, so the reversed
            # pair-scan auto-resets at the tile boundary).  The cp-pair
            # table is written in swapped order [cp_rev(t+1) | cp_rev(t)]
            # to match the reversed stream.
            NP = NT // 2
            xo_p = xout.rearrange("(t p) j -> p t j", p=128)
            tabcs = [None] * NP
            ets = [None] * NP
            dps = [None] * NP

            def stage_front(p):
                t = 2 * p
                for ti in (t, t + 1):
                    ptsl = pts[:, ti * 128:(ti + 1) * 128]
                    taba = tabapool.tile([128, NV], F32, tag="ta")
                    nc.tensor.matmul(taba[:], ptsl, kt[:, 0:NV],
                                     start=True, stop=True)
                    tabb = tabbpool.tile([128, NV], F32, tag="tb")
                    nc.tensor.matmul(tabb[:], ptsl, kt[:, NV:2 * NV],
                                     start=True, stop=True)
                    if ti == t:
                        tabc = tabcpool.tile([128, 2 * NV], F32, tag="tc")
                        et = epool.tile([128, 2 * NV], F32, tag="e")
                        dpt = dppool.tile([128, 2 * NV], F32, tag="dp")
                    half = slice((ti - t) * NV, (ti - t + 1) * NV)
                    chalf = slice((t + 1 - ti) * NV, (t + 2 - ti) * NV)
                    nc.tensor.matmul(tabc[:, chalf], ptsl,
                                     kt[:, 2 * NV:3 * NV],
                                     start=True, stop=True)
                    bsb = epool.tile([128, NV], F32, tag="bsb")
                    nc.scalar.copy(bsb[:], tabb[:])
                    nc.vector.tensor_tensor_scan(
                        out=et[:, half], data0=taba[:],
                        data1=fall[:, ti * NV:(ti + 1) * NV],
                        initial=0.0, op0=ALU.mult, op1=ALU.add)
                    # premultiply offloaded to the otherwise-idle GpSimd
                    nc.gpsimd.tensor_tensor(dpt[:, half], et[:, half],
                                            bsb[:], ALU.mult)
                tabcs[p], ets[p], dps[p] = tabc, et, dpt

            def stage_back(p):
                t = 2 * p
                dpt = dps[p]
                xt = xpool.tile([128, 2 * NV], F32, tag="x")
                nc.vector.tensor_tensor_scan(
                    out=xt[:, ::-1], data0=tabcs[p][:],
                    data1=dpt[:, ::-1], initial=0.0, op0=ALU.mult,
                    op1=ALU.add)
                if p == NP - 1:
                    nc.scalar.dma_start(xo_p[:, t:t + 1, :], xt[:, 0:NV])
                    nc.sync.dma_start(xo_p[:, t + 1:t + 2, :],
                                      xt[:, NV:2 * NV])
                else:
                    nc.scalar.dma_start(xo_p[:, t:t + 2, :], xt[:])

            stage_front(0)
            for p in range(NP):
                if p + 1 < NP:
                    stage_front(p + 1)
                stage_back(p)

    nc.compile()
    return nc


_PROGRAM_CACHE = {}


def _get_program():
    if "prog" not in _PROGRAM_CACHE:
        _PROGRAM_CACHE["prog"] = build_program()
    return _PROGRAM_CACHE["prog"]


def make_in_maps(f0x, dt, v):
    f0x = np.ascontiguousarray(np.asarray(f0x, np.float32))
    v = np.asarray(v, np.float32)
    ktab, pstack = _build_tables_and_basis(f0x, float(dt), v)
    in_maps = []
    for c in range(N_CORES):
        shard = f0x[c * ROWS:(c + 1) * ROWS]
        ps = pstack[c * ROWS:(c + 1) * ROWS]         # [ROWS, 24]
        ptin = np.ascontiguousarray(ps.reshape(NT, 128, KS)
                                    .transpose(2, 0, 1).reshape(KS, NT * 128))
        in_maps.append({
            "fin": np.ascontiguousarray(shard),
            "ktab": ktab, "ptin": ptin,
        })
    return in_maps


def kernel(nu, f0x, dt, v):
    import os
    import time
    nc = _get_program()
    in_maps = make_in_maps(f0x, dt, v)
    trace = bool(os.environ.get("KERNEL_TRACE"))
    res = None
    last_exc = None
    for attempt in range(3):
        try:
            res = run_bass_kernel_spmd(nc, in_maps,
                                       core_ids=list(range(N_CORES)),
                                       trace=trace)
            break
        except Exception as e:   # transient device wedges have been observed
            last_exc = e
            time.sleep(5.0 * (attempt + 1))
    if res is None:
        raise last_exc
    if trace:
        kernel.last_results = res
    out = np.concatenate([r["xout"] for r in res.results], axis=0)
    return out.astype(np.float32)
